# revision 3
# baseline (speedup 1.0000x reference)
"""GatedAttMIL segment-softmax pooling kernel for 8x TRN2 NeuronCores.

Math (per reference):
    A = tanh(feats @ Vw.T + Vb) * sigmoid(feats @ Uw.T + Ub)   # (N, 128)
    s = A @ ww.T                                                # (N,)
    out[g] = sum_{i: idx_i=g} softmax-weight_i * feats[i]       # (G, D)

Key observations exploited here:
  * |s| <= ||ww||_1 (~9 for this data) since |tanh*sigmoid| < 1, so exp(s)
    cannot overflow fp32 and the segment-max subtraction is unnecessary:
    out[g] = (sum e^{s_i} f_i) / (sum e^{s_i}).  Partial numerator/denominator
    sums are then exactly additive across cores -> no collectives; the host
    adds the per-core partials for boundary groups.
  * index is sorted, so a contiguous shard of N/8 = 32768 rows spans < 128
    distinct groups.  Using local group ids (index - first index of shard),
    a single 128-wide one-hot matmul accumulates the pooled output.
  * feats ship to HBM as bf16 (tolerance is 2e-2; bf16 adds ~4e-3): halves
    the DMA floor.  Row->partition mapping is (p t) so each partition's
    source bytes are one contiguous 8KB run per 1024-row superblock.
  * PE budget: transposes run on bf16 data with a bf16 identity (1 cy/row),
    V/U projections use fp8e4 operands in DoubleRow perf mode (0.5 cy/row,
    two 128-d-chunks per instruction), pooled matmuls use bf16 one-hot x
    bf16 feats (1 cy/row vs 4 for fp32).
  * pooled/denom matmuls for block b issue after block b+1's score matmuls
    (software pipelining) so the PE never stalls on the ACT/DVE score chain.

Per-core dataflow (one 512-row block at a time):
  DMA bf16 feats superblock -> PE transpose (bf16 identity) -> PSUM bf16 ->
  copy-out to fp8 xT (spread over DVE/ACT/Pool) -> V/U DoubleRow matmuls
  with stationary fp8 VwT/UwT -> tanh / sigmoid-via-tanh on ACT ->
  A = tv*tu on DVE -> per-row scores via A-stationary matmul against ww ->
  exp on ACT -> OHW[i,g] = (iota==lidx_i)*e_i fused on Pool ->
  pooled[g,:] += OHW^T @ feats_block, denom[g] += OHW^T @ 1 in PSUM.
"""

import os

import numpy as np

P = 128          # partitions
N = 262144       # instances
D = 512          # feature dim
DA = 128         # attention dim
G = 512          # num groups
N_CORES = 8
SHARD = N // N_CORES          # 32768 rows per core
TILES = SHARD // P            # 256 tiles of 128 rows
TPB = 4                       # tiles per block
BLOCKS = TILES // TPB         # 64 blocks of 512 rows
TPS = 8                       # tiles per superblock (1024 rows, one DMA)
NSB = TILES // TPS            # 32 superblocks

FP8_VU = True                 # fp8e4 DoubleRow for the V/U projections

_CACHE = {}

# test.py reads this after calling kernel() to get exec_time_ns / trace info
last_results = None


def _build():
    import concourse.bacc as bacc
    import concourse.mybir as mybir
    import concourse.tile as tile
    from concourse.masks import make_identity

    f32 = mybir.dt.float32
    bf16 = mybir.dt.bfloat16
    f8 = mybir.dt.float8e4
    xt_dt = f8 if FP8_VU else bf16
    AF = mybir.ActivationFunctionType
    ALU = mybir.AluOpType
    DR = mybir.MatmulPerfMode.DoubleRow

    nc = bacc.Bacc("TRN2", target_bir_lowering=False, debug=False,
                   num_devices=N_CORES)

    x_d = nc.dram_tensor("x", [SHARD, D], bf16, kind="ExternalInput").ap()
    lidx_d = nc.dram_tensor("lidxT", [P, TILES], f32, kind="ExternalInput").ap()
    vw_d = nc.dram_tensor("vw", [P, 4, DA], xt_dt, kind="ExternalInput").ap()
    uw_d = nc.dram_tensor("uw", [P, 4, DA], xt_dt, kind="ExternalInput").ap()
    vb_d = nc.dram_tensor("vb", [P, 1], f32, kind="ExternalInput").ap()
    ubh_d = nc.dram_tensor("ubh", [P, 1], f32, kind="ExternalInput").ap()
    ww_d = nc.dram_tensor("wwt", [P, 1], bf16, kind="ExternalInput").ap()
    iota_d = nc.dram_tensor("iota", [P, P], bf16, kind="ExternalInput").ap()
    pooled_d = nc.dram_tensor("pooled", [P, D], f32, kind="ExternalOutput").ap()
    denom_d = nc.dram_tensor("denom", [P, 1], f32, kind="ExternalOutput").ap()

    with tile.TileContext(nc) as tc:
        with (
            tc.tile_pool(name="const", bufs=1) as cp,
            tc.tile_pool(name="sb", bufs=3) as sb,
            tc.tile_pool(name="ps", bufs=1, space="PSUM") as pp,
        ):
            ident = cp.tile([P, P], bf16)
            make_identity(nc, ident[:])
            vw_s = cp.tile([P, 4, DA], xt_dt)
            nc.sync.dma_start(out=vw_s[:], in_=vw_d)
            uw_s = cp.tile([P, 4, DA], xt_dt)
            nc.sync.dma_start(out=uw_s[:], in_=uw_d)
            vb_s = cp.tile([P, 1], f32)
            nc.sync.dma_start(out=vb_s[:], in_=vb_d)
            ubh_s = cp.tile([P, 1], f32)
            nc.sync.dma_start(out=ubh_s[:], in_=ubh_d)
            ww_s = cp.tile([P, 1], bf16)
            nc.sync.dma_start(out=ww_s[:], in_=ww_d)
            iota_s = cp.tile([P, P], bf16)
            nc.sync.dma_start(out=iota_s[:], in_=iota_d)
            lidx_s = cp.tile([P, TILES], f32)
            nc.sync.dma_start(out=lidx_s[:], in_=lidx_d)
            ones_s = cp.tile([P, 1], bf16)
            nc.vector.memset(ones_s[:], 1.0)

            # persistent accumulators (1 PSUM bank each, live whole kernel)
            pooled_ps = pp.tile([P, D], f32, tag="pooled")
            denom_ps = pp.tile([P, 1], f32, tag="denom")

            # deferred pooled/denom matmuls: list of (gt, ohw_ap, x_ap)
            pend = []
            n_flushed = [0]

            def flush_pend():
                for gt, ohw_ap, x_ap in pend:
                    nc.tensor.matmul(
                        out=pooled_ps[:], lhsT=ohw_ap, rhs=x_ap,
                        start=(gt == 0), stop=(gt == TILES - 1))
                    nc.tensor.matmul(
                        out=denom_ps[:], lhsT=ohw_ap, rhs=ones_s[:],
                        start=(gt == 0), stop=(gt == TILES - 1))
                    n_flushed[0] += 1
                pend.clear()

            def block(b, xb_s, half):
                """Process one 512-row block: tiles half*4 .. half*4+3."""
                # transpose to xT (cast to fp8 in the PSUM->SBUF copy-out),
                # chunk-major: xT_s[:, c, :] = [d=c*128+p, i]
                xT_s = sb.tile([P, 4, D], xt_dt, tag="xT", bufs=2,
                               name=f"xT_{b}")
                # gpsimd (Pool) cannot access PSUM -> only DVE/ACT copy out
                coeng = [nc.vector, nc.scalar, nc.vector, nc.scalar]
                for c in range(4):
                    xt_ps = pp.tile([P, D], bf16, tag="xt", bufs=2,
                                    name=f"xtp_{b}_{c}")
                    for t in range(TPB):
                        nc.tensor.transpose(
                            out=xt_ps[:, t * P:(t + 1) * P],
                            in_=xb_s[:, half * TPB + t, c * P:(c + 1) * P],
                            identity=ident[:],
                        )
                    eng = coeng[c]
                    if eng is nc.scalar:
                        eng.copy(out=xT_s[:, c, :], in_=xt_ps[:])
                    else:
                        eng.tensor_copy(out=xT_s[:, c, :], in_=xt_ps[:])

                # V/U projections over 4 d-chunks
                v_ps = pp.tile([P, D], f32, tag="v", bufs=1, name=f"v_{b}")
                u_ps = pp.tile([P, D], f32, tag="u", bufs=1, name=f"u_{b}")
                if FP8_VU:
                    # DoubleRow: two 128-d-chunks per matmul at 0.5 cy/row
                    for pr in range(2):
                        nc.tensor.matmul(
                            out=v_ps[:], lhsT=vw_s[:, 2 * pr:2 * pr + 2, :],
                            rhs=xT_s[:, 2 * pr:2 * pr + 2, :],
                            start=(pr == 0), stop=(pr == 1), perf_mode=DR)
                    for pr in range(2):
                        nc.tensor.matmul(
                            out=u_ps[:], lhsT=uw_s[:, 2 * pr:2 * pr + 2, :],
                            rhs=xT_s[:, 2 * pr:2 * pr + 2, :],
                            start=(pr == 0), stop=(pr == 1), perf_mode=DR)
                else:
                    for c in range(4):
                        nc.tensor.matmul(
                            out=v_ps[:], lhsT=vw_s[:, c, :],
                            rhs=xT_s[:, c, :],
                            start=(c == 0), stop=(c == 3))
                    for c in range(4):
                        nc.tensor.matmul(
                            out=u_ps[:], lhsT=uw_s[:, c, :],
                            rhs=xT_s[:, c, :],
                            start=(c == 0), stop=(c == 3))

                # tv = tanh(v + Vb); tu = sigmoid(u + Ub) via
                # sigmoid(x) = 0.5*(1 + tanh(x/2)): one ACT table set
                tv_s = sb.tile([P, D], bf16, tag="tv", name=f"tv_{b}")
                nc.scalar.activation(out=tv_s[:], in_=v_ps[:], func=AF.Tanh,
                                     bias=vb_s[:, 0:1], scale=1.0)
                tu_s = sb.tile([P, D], bf16, tag="tu", name=f"tu_{b}")
                nc.scalar.activation(out=tu_s[:], in_=u_ps[:], func=AF.Tanh,
                                     bias=ubh_s[:, 0:1], scale=0.5)
                nc.vector.tensor_scalar(out=tu_s[:], in0=tu_s[:],
                                        scalar1=0.5, scalar2=0.5,
                                        op0=ALU.mult, op1=ALU.add)
                a_s = sb.tile([P, D], bf16, tag="a", name=f"a_{b}")
                nc.vector.tensor_tensor(out=a_s[:], in0=tv_s[:], in1=tu_s[:],
                                        op=ALU.mult)

                # per-row scores: s[i] = sum_a A[a, i] * ww[a]
                sc_ps = pp.tile([P, TPB], f32, tag="sc", name=f"sc_{b}")
                for t in range(TPB):
                    nc.tensor.matmul(
                        out=sc_ps[:, t:t + 1],
                        lhsT=a_s[:, t * P:(t + 1) * P], rhs=ww_s[:],
                        start=(t == 0), stop=(t == TPB - 1))

                # previous block's pooled/denom land here: the PE chews on
                # them while ACT/Pool produce this block's e and one-hots
                flush_pend()

                e_s = sb.tile([P, TPB], f32, tag="e", name=f"e_{b}")
                nc.scalar.activation(out=e_s[:], in_=sc_ps[:], func=AF.Exp)

                # weighted one-hot; pooled/denom matmuls are deferred
                for t in range(TPB):
                    gt = b * TPB + t
                    ohw_s = sb.tile([P, P], bf16, tag="ohw", bufs=10,
                                    name=f"ohw_{gt}")
                    nc.gpsimd.tensor_scalar(
                        out=ohw_s[:], in0=iota_s[:],
                        scalar1=lidx_s[:, gt:gt + 1],
                        scalar2=e_s[:, t:t + 1],
                        op0=ALU.is_equal, op1=ALU.mult)
                    pend.append((gt, ohw_s[:],
                                 xb_s[:, half * TPB + t, :]))

            for sbk in range(NSB):
                # rows [sbk*1024, (sbk+1)*1024): partition p sources rows
                # 8p..8p+7, i.e. one contiguous 8KB HBM run per partition
                xb_s = sb.tile([P, TPS, D], bf16, tag="x", bufs=2,
                               name=f"x_{sbk}")
                nc.sync.dma_start(
                    out=xb_s[:],
                    in_=x_d[sbk * 1024:(sbk + 1) * 1024, :].rearrange(
                        "(p t) d -> p t d", t=TPS),
                )
                for half in range(2):
                    block(2 * sbk + half, xb_s, half)
            flush_pend()
            assert n_flushed[0] == TILES

            pooled_s = sb.tile([P, D], f32, tag="outp")
            nc.vector.tensor_copy(out=pooled_s[:], in_=pooled_ps[:])
            nc.sync.dma_start(out=pooled_d, in_=pooled_s[:])
            denom_s = sb.tile([P, 1], f32, tag="outd")
            nc.vector.tensor_copy(out=denom_s[:], in_=denom_ps[:])
            nc.sync.dma_start(out=denom_d, in_=denom_s[:])

    nc.compile()
    return nc


def prepare_in_maps(feats, index, num_groups, Vw, Vb, Uw, Ub, ww):
    """Host-side prep: per-core input dicts + shard group offsets."""
    feats = np.ascontiguousarray(np.asarray(feats, dtype=np.float32))
    index = np.asarray(index)
    Vw = np.asarray(Vw, dtype=np.float32)
    Vb = np.asarray(Vb, dtype=np.float32)
    Uw = np.asarray(Uw, dtype=np.float32)
    Ub = np.asarray(Ub, dtype=np.float32)
    ww = np.asarray(ww, dtype=np.float32)

    import ml_dtypes
    bf16 = ml_dtypes.bfloat16
    f8 = ml_dtypes.float8_e4m3
    wdt = f8 if FP8_VU else bf16

    feats_bf = feats.astype(bf16)

    # chunk-major transposed weights: w3[p, c, a] = W[a, c*128 + p]
    def chunkT(w):  # (DA, D) -> (P, 4, DA)
        return np.ascontiguousarray(
            w.T.reshape(4, P, DA).transpose(1, 0, 2)).astype(wdt)

    vw3 = chunkT(Vw)
    uw3 = chunkT(Uw)
    vb = np.ascontiguousarray(Vb.reshape(P, 1))
    ubh = np.ascontiguousarray(0.5 * Ub.reshape(P, 1))
    wwt = np.ascontiguousarray(ww.reshape(DA, 1).astype(bf16))
    iota = np.ascontiguousarray(
        np.broadcast_to(np.arange(P, dtype=np.float32), (P, P))).astype(bf16)

    g_starts = []
    in_maps = []
    for c in range(N_CORES):
        sl = slice(c * SHARD, (c + 1) * SHARD)
        g0 = int(index[c * SHARD])
        g_starts.append(g0)
        lidx = (index[sl].astype(np.int64) - g0)
        assert lidx.min() >= 0 and lidx.max() < P, (
            f"core {c}: shard spans {lidx.max() + 1} groups (>128)")
        # row (sb*1024 + 8p + t) sits at partition p, tile gt = sb*8 + t
        lidxT = np.ascontiguousarray(
            lidx.astype(np.float32).reshape(NSB, P, TPS)
            .transpose(1, 0, 2).reshape(P, TILES))
        in_maps.append({
            "x": feats_bf[sl],
            "lidxT": lidxT,
            "vw": vw3, "uw": uw3, "vb": vb, "ubh": ubh, "wwt": wwt,
            "iota": iota,
        })
    return in_maps, g_starts


def merge(results, g_starts, G_):
    """Combine per-core partial (pooled, denom) into the global output."""
    num = np.zeros((G_, D), np.float64)
    den = np.zeros((G_,), np.float64)
    for c in range(N_CORES):
        g0 = g_starts[c]
        nrows = min(P, G_ - g0)
        num[g0:g0 + nrows] += results[c]["pooled"][:nrows].astype(np.float64)
        den[g0:g0 + nrows] += results[c]["denom"][:nrows, 0].astype(np.float64)
    safe = np.maximum(den, 1e-300)
    out = np.where(den[:, None] > 0.0, num / safe[:, None], 0.0)
    return out.astype(np.float32)


def kernel(feats, index, num_groups, Vw, Vb, Uw, Ub, ww):
    global last_results
    from concourse.bass_utils import run_bass_kernel_spmd

    G_ = int(num_groups)
    in_maps, g_starts = prepare_in_maps(feats, index, num_groups,
                                        Vw, Vb, Uw, Ub, ww)

    if "nc" not in _CACHE:
        _CACHE["nc"] = _build()
    nc = _CACHE["nc"]

    res = run_bass_kernel_spmd(
        nc, in_maps, core_ids=list(range(N_CORES)),
        trace=bool(os.environ.get("BASS_TRACE")),
    )
    last_results = res
    return merge([res.results[c] for c in range(N_CORES)], g_starts, G_)
